# revision 17
# baseline (speedup 1.0000x reference)
"""Bass/Trainium2 kernel for nn_BiLSTMDecoderModel (BiLSTM encoder + GRU decoder).

Contract: kernel(**inputs) takes the FULL unsharded inputs (as produced by
reference.setup_inputs()) and returns the FULL [C, B, 2] log-softmax output.

Strategy (8 NeuronCores, SPMD):
  - Data-parallel over batch: each core owns B/8 = 16 sequences and runs the
    full model on its slice (identical program, per-core input data).
  - Embedding rows are gathered on-chip via indirect DMA (bf16 table), PE-
    transposed to [E, token] layout, tanh'd into SBUF.  The backward
    direction reads the same buffer through negative-stride APs (no mirrored
    copy).
  - Input projections (x @ W_ih^T + biases) are bulk matmuls writing directly
    into per-step PSUM regions; the recurrent matmuls (h @ W_hh^T) then
    accumulate into the same regions, so gate pre-activations never round-trip
    through SBUF/HBM.
  - LSTM cell per step per direction (critical serial chain, minimized op
    count):  gates permuted host-side to (f, i, o, g) with g-rows scaled by
    2 so ONE sigmoid ACT covers all 8 gate chunks (tanh g = 2*sigmoid(2g)-1);
    the [c | g^] pair is kept adjacent in SBUF so the two c-products fuse
    into one wide tensor_tensor; the two directions' chains are emitted
    zip-interleaved so the engine FIFOs anti-phase them.
  - 6-step GRU decoder + projection + classifier + log-softmax run per core on
    its own batch slice; no collectives anywhere.
"""

import os
import sys

import numpy as np

for _p in ("/opt/trn_rl_repo",):
    if os.path.isdir(_p) and _p not in sys.path:
        sys.path.insert(0, _p)

import ml_dtypes
from contextlib import ExitStack

from concourse import bass, bacc, mybir, tile
from concourse.bass_utils import run_bass_kernel_spmd
from concourse.masks import make_identity
from concourse.tile_rust import add_dep_helper

BF16 = ml_dtypes.bfloat16
F32 = np.float32

V, C, E, H, PP = 100000, 6, 300, 256, 256
B, S = 128, 512
NCORES = 8
BPC = B // NCORES  # 16 sequences per core

EK = 3   # ceil((E+1)/128) chunks of the (augmented) embedding dim
G4 = 8   # 4H / 128 gate chunks
HK = 2   # H / 128 chunks
DG = 12  # 3*2H / 128 decoder gate chunks
DK = 4   # 2H / 128 decoder hidden chunks
PK = 2   # P / 128 proj chunks
BIAS_ROW = 96  # chunk-2 partition of the augmented "1" (bias) row

_FT = mybir.ActivationFunctionType
_ALU = mybir.AluOpType

_BUILD_CACHE = {}


def _pack_kxm(wt, kchunks, mchunks):
    """[kchunks*128, mchunks*128] -> [128, kchunks, mchunks, 128] tile pack."""
    a = wt.reshape(kchunks, 128, mchunks, 128).transpose(1, 0, 2, 3)
    return np.ascontiguousarray(a.astype(BF16))


def _aug_wihT(Wih, bias, mchunks):
    """W_ih [4H, E] + bias [4H] -> augmented, padded [EK*128, 4H] transpose."""
    out = np.zeros((EK * 128, Wih.shape[0]), dtype=F32)
    out[:E] = Wih.T.astype(F32)
    out[2 * 128 + BIAS_ROW] = bias.astype(F32)
    return _pack_kxm(out, EK, mchunks)


def _build_program(s_steps):
    """Build the SPMD Bass program (one NeuronCore's view). Returns nc."""
    SS = s_steps
    NT = SS * BPC // 128          # number of 128-token gather tiles
    NBANK = SS // 4               # psum gx banks per direction
    assert SS % 8 == 0

    nc = bacc.Bacc("TRN2", target_bir_lowering=False, debug=False,
                   num_devices=NCORES)
    dt = mybir.dt

    # ---- DRAM I/O ----
    seqi = nc.declare_dram_parameter("seqi", [128, NT], dt.int32, isOutput=False)
    emb = nc.declare_dram_parameter("emb", [V, E], dt.bfloat16, isOutput=False)
    wih = {d: nc.declare_dram_parameter(f"wih_{d}", [128, EK, G4, 128],
                                        dt.bfloat16, isOutput=False)
           for d in "fb"}
    whh = {d: nc.declare_dram_parameter(f"whh_{d}", [128, HK, G4, 128],
                                        dt.bfloat16, isOutput=False)
           for d in "fb"}
    dwih = nc.declare_dram_parameter("dwih", [128, EK, DG, 128], dt.bfloat16,
                                     isOutput=False)
    dwhh = nc.declare_dram_parameter("dwhh", [128, DK, DG, 128], dt.bfloat16,
                                     isOutput=False)
    bhhn = nc.declare_dram_parameter("bhhn", [128, DK, 1], dt.float32,
                                     isOutput=False)  # [128, 4, 1] n-gate bhh
    pw = nc.declare_dram_parameter("pw", [128, DK, PK, 128], dt.bfloat16,
                                   isOutput=False)
    pb = nc.declare_dram_parameter("pb", [128, PK], dt.float32, isOutput=False)
    cw = nc.declare_dram_parameter("cw", [128, PK, 2], dt.bfloat16,
                                   isOutput=False)
    cb = nc.declare_dram_parameter("cb", [128, 2], dt.float32, isOutput=False)
    ecw = nc.declare_dram_parameter("ecw", [C, E], dt.bfloat16, isOutput=False)
    clsi = nc.declare_dram_parameter("clsi", [C, 1], dt.int32, isOutput=False)
    y = nc.declare_dram_parameter("y", [C * BPC, 2], dt.float32, isOutput=True)

    with tile.TileContext(nc) as tc, ExitStack() as ctx:
        # ---- long-lived SBUF ----
        const = ctx.enter_context(tc.tile_pool(name="const", bufs=1))
        ident = const.tile([128, 128], dt.bfloat16, tag="ident")
        make_identity(nc, ident[:])
        seqi_sb = const.tile([128, NT], dt.int32, tag="seqi")
        nc.sync.dma_start(out=seqi_sb[:], in_=seqi[:])
        wih_sb = {}
        whh_sb = {}
        for d in "fb":
            wih_sb[d] = const.tile([128, EK, G4, 128], dt.bfloat16,
                                   tag=f"wih{d}", name=f"wih_sb_{d}")
            nc.sync.dma_start(out=wih_sb[d][:], in_=wih[d][:])
            whh_sb[d] = const.tile([128, HK, G4, 128], dt.bfloat16,
                                   tag=f"whh{d}", name=f"whh_sb_{d}")
            nc.sync.dma_start(out=whh_sb[d][:], in_=whh[d][:])
        # single tanh'd-embedding buffer; backward reads it via negative
        # strides.  zero the pad rows of chunk 2 once; set the bias row.
        xT = const.tile([128, EK, SS, BPC], dt.bfloat16, tag="xT")
        nc.vector.memset(xT[:, EK - 1, :, :], 0.0)
        nc.vector.memset(xT[BIAS_ROW:BIAS_ROW + 1, EK - 1, :, :], 1.0)

        dec = ctx.enter_context(tc.tile_pool(name="dec", bufs=1))
        dwih_sb = dec.tile([128, EK, DG, 128], dt.bfloat16, tag="dwih")
        nc.sync.dma_start(out=dwih_sb[:], in_=dwih[:])
        dwhh_sb = dec.tile([128, DK, DG, 128], dt.bfloat16, tag="dwhh")
        nc.sync.dma_start(out=dwhh_sb[:], in_=dwhh[:])
        bhhn_sb = dec.tile([128, DK, 1], dt.float32, tag="bhhn")
        nc.sync.dma_start(out=bhhn_sb[:], in_=bhhn[:])
        pw_sb = dec.tile([128, DK, PK, 128], dt.bfloat16, tag="pw")
        nc.sync.dma_start(out=pw_sb[:], in_=pw[:])
        pb_sb = dec.tile([128, PK], dt.float32, tag="pb")
        nc.sync.dma_start(out=pb_sb[:], in_=pb[:])
        cw_sb = dec.tile([128, PK, 2], dt.bfloat16, tag="cw")
        nc.sync.dma_start(out=cw_sb[:], in_=cw[:])
        cb_sb = dec.tile([128, 2], dt.float32, tag="cb")
        nc.sync.dma_start(out=cb_sb[:], in_=cb[:])
        clsi_sb = dec.tile([C, 1], dt.int32, tag="clsi")
        nc.sync.dma_start(out=clsi_sb[:], in_=clsi[:])
        ce = dec.tile([C, E], dt.bfloat16, tag="ce")
        nc.gpsimd.indirect_dma_start(
            out=ce[:], out_offset=None, in_=ecw[:],
            in_offset=bass.IndirectOffsetOnAxis(ap=clsi_sb[:, :1], axis=0))

        # ---- pipelined pools ----
        rec_ctx = ExitStack()
        gath = rec_ctx.enter_context(tc.tile_pool(name="gath", bufs=10))
        tp_ps = rec_ctx.enter_context(
            tc.tile_pool(name="tp", bufs=2, space="PSUM"))
        gxp = {d: rec_ctx.enter_context(
            tc.tile_pool(name=f"gx{d}", bufs=3, space="PSUM")) for d in "fb"}
        sigp = {d: rec_ctx.enter_context(
            tc.tile_pool(name=f"sig{d}", bufs=3)) for d in "fb"}
        t12p = {d: rec_ctx.enter_context(
            tc.tile_pool(name=f"t12{d}", bufs=3)) for d in "fb"}
        tcp = {d: rec_ctx.enter_context(
            tc.tile_pool(name=f"tc{d}", bufs=3)) for d in "fb"}
        cgp = {d: rec_ctx.enter_context(
            tc.tile_pool(name=f"cg{d}", bufs=3)) for d in "fb"}
        hstp = {d: rec_ctx.enter_context(
            tc.tile_pool(name=f"hst{d}", bufs=3)) for d in "fb"}

        def gather_dma(g):
            """Start the indirect gather DMA for tile g; returns the buffer."""
            gt = gath.tile([128, E], dt.bfloat16, tag="g")
            nc.gpsimd.indirect_dma_start(
                out=gt[:], out_offset=None, in_=emb[:],
                in_offset=bass.IndirectOffsetOnAxis(ap=seqi_sb[:, g:g + 1],
                                                    axis=0))
            return gt

        def gather_transpose(gt, g, k, anchor=None):
            """PE-transpose chunk k of gathered tile g; returns psum tile."""
            lo = k * 128
            w = min(E, lo + 128) - lo
            tp = tp_ps.tile([128, 128], dt.bfloat16, space="PSUM", tag="tp")
            bi = nc.tensor.transpose(out=tp[0:w, :], in_=gt[:, lo:lo + w],
                                     identity=ident[:])
            if anchor is not None:
                # ordering-only edge: stop the static scheduler hoisting this
                # into the PE stream ahead of the recurrence bursts
                add_dep_helper(bi.ins, anchor, sync=False,
                               reason="keep filler PE work behind the chain")
            return tp

        def gather_tanh(tp, g, k):
            """tanh chunk k of tile g from psum into xT."""
            lo = k * 128
            w = min(E, lo + 128) - lo
            t0 = g * (128 // BPC)
            nsub = 128 // BPC
            nc.scalar.activation(
                xT[0:w, k, t0:t0 + nsub, :], tp[0:w, :], _FT.Tanh)

        def gather_tile(g):
            gt = gather_dma(g)
            for k in range(EK):
                gather_tanh(gather_transpose(gt, g, k), g, k)

        def gx_alloc(d, j, banks):
            bank = gxp[d].tile([128, G4, 4, BPC], dt.float32, space="PSUM",
                               tag=f"gxb{d}")
            # psum start=True lazily zeroes the WHOLE 2KB bank, so exactly one
            # matmul (the first) starts the group; ordering edges keep every
            # other matmul after it.  The group is closed by the last
            # recurrence matmul of the bank's 4th step.
            banks[j] = (bank, [None])

        def gx_mms(d, j, banks, mlo, mhi, anchor=None):
            """Input-projection matmuls for m-chunks [mlo, mhi) of bank j.

            Backward direction: step u reads token (SS-u) % SS, i.e. bank j
            covers tokens SS-4j .. SS-4j-3 -> negative-stride slices of xT.
            """
            bank, firstbox = banks[j]

            def _mm(out, lhsT, rhs):
                bi = nc.tensor.matmul(out=out, lhsT=lhsT, rhs=rhs,
                                      start=(firstbox[0] is None), stop=False)
                if firstbox[0] is None:
                    firstbox[0] = bi.ins
                else:
                    add_dep_helper(bi.ins, firstbox[0], sync=False,
                                   reason="psum bank single-start order")
                if anchor is not None:
                    add_dep_helper(bi.ins, anchor, sync=False,
                                   reason="keep filler PE work behind the chain")

            for m in range(mlo, mhi):
                for k in range(EK):
                    lhsT = wih_sb[d][:, k, m, :]
                    if d == "f":
                        _mm(bank[:, m, :, :], lhsT, xT[:, k, 4 * j:4 * j + 4, :])
                    elif j == 0:
                        _mm(bank[:, m, 0, :], lhsT, xT[:, k, 0, :])
                        _mm(bank[:, m, 1:4, :], lhsT,
                            xT[:, k, SS - 1:SS - 4:-1, :])
                    else:
                        _mm(bank[:, m, :, :], lhsT,
                            xT[:, k, SS - 4 * j:SS - 4 * j - 4:-1, :])

        banks = {"f": {}, "b": {}}
        cg_st = {}
        h_st = {"f": None, "b": None}
        for d in "fb":
            cg0 = cgp[d].tile([128, 4, BPC], dt.bfloat16, tag=f"cg{d}")
            nc.vector.memset(cg0[:, 0:2, :], 0.0)
            cg_st[d] = cg0

        # prologue: front tiles 0-3 and back tiles NT-1..NT-4 in full
        # (all DMAs in flight first); gx banks 0,1 per dir.
        NPRO = 4
        pro_tiles = []
        for i in range(NPRO):
            pro_tiles.append(i)
            pro_tiles.append(NT - 1 - i)
        pro_bufs = [gather_dma(g) for g in pro_tiles]
        for g, gt in zip(pro_tiles, pro_bufs):
            for k in range(EK):
                gather_tanh(gather_transpose(gt, g, k), g, k)
        for j in (0, 1):
            for d in "fb":
                gx_alloc(d, j, banks[d])
                gx_mms(d, j, banks[d], 0, G4)

        # steady-state gather schedule: one chunk (transpose+tanh) per step,
        # one tile per 3 steps, alternating front/back.
        gorder = []
        fr, bk = NPRO, NT - 1 - NPRO
        while fr <= bk:
            gorder.append(fr)
            fr += 1
            if fr <= bk:
                gorder.append(bk)
                bk -= 1
        gbuf = {}
        pend_tanh = []
        anchor = None

        for t in range(SS):
            # ---- PE work first: recurrence matmuls for both directions ----
            for d in "fb":
                bank = banks[d][t // 4][0]
                reg = bank[:, :, t % 4, :]
                morder = (6, 7, 0, 1, 2, 3, 4, 5) if d == "f" else range(G4)
                if t > 0:
                    h = h_st[d]
                    for mi, m in enumerate(morder):
                        for k in range(HK):
                            last = (t % 4 == 3 and mi == G4 - 1
                                    and k == HK - 1)
                            bi = nc.tensor.matmul(
                                out=reg[:, m, :], lhsT=whh_sb[d][:, k, m, :],
                                rhs=h[:, k, :], start=False, stop=last)
                            if d == "f" and mi == 0 and k == 0:
                                anchor = bi.ins
            # ---- PE filler: spread next gx bank (2 m-chunks per step) ----
            j = t // 4 + 2
            if j < NBANK:
                if t % 4 == 0:
                    for d in "fb":
                        gx_alloc(d, j, banks[d])
                mlo = 2 * (t % 4)
                for d in "fb":
                    gx_mms(d, j, banks[d], mlo, mlo + 2, anchor=anchor)
            # ---- spread gathers: 1 chunk per step, DMA 2 steps ahead ----
            gi, gk = t // 3, t % 3
            if gi < len(gorder):
                if gk == 0 and gi not in gbuf:
                    gbuf[gi] = gather_dma(gorder[gi])
                pend_tanh.append((gather_transpose(gbuf[gi], gorder[gi], gk,
                                                   anchor=anchor),
                                  gorder[gi], gk, gi))
                if gk == 2 and gi + 1 < len(gorder):
                    gbuf[gi + 1] = gather_dma(gorder[gi + 1])

            # ---- cell chains; f first, b lags (FIFO anti-phase) ----
            # gate chunk order (f, i, o, g~): one sigmoid covers all 8 chunks
            # (g rows pre-scaled by 2 -> tanh g = 2*sigmoid(2g) - 1).
            hf_ins = None
            tt2f_ins = None
            tcf_ins = None
            for d in "fb":
                bank_reg = banks[d][t // 4][0][:, :, t % 4, :]
                if d == "f":
                    # tanh(g) directly from psum into the slot adjacent to
                    # c; runs on ACT inside the matmul window (g MMs first)
                    nc.scalar.activation(cg_st[d][:, 2:4, :],
                                         bank_reg[:, 6:8, :], _FT.Tanh)
                    sg = sigp[d].tile([128, 6, BPC], dt.bfloat16,
                                      tag=f"sig{d}")
                    nc.scalar.activation(sg[:], bank_reg[:, 0:6, :],
                                         _FT.Sigmoid)
                else:
                    sg = sigp[d].tile([128, G4, BPC], dt.bfloat16,
                                      tag=f"sig{d}")
                    bi_sg = nc.scalar.activation(sg[:], bank_reg[:],
                                                 _FT.Sigmoid)
                    if tcf_ins is not None:
                        add_dep_helper(bi_sg.ins, tcf_ins, sync=False,
                                       reason="sigma_b after tanhc_f")
                    # g^ = 2*sigmoid(2g) - 1 into the slot adjacent to c
                    bi_ts = nc.vector.tensor_scalar(
                        out=cg_st[d][:, 2:4, :], in0=sg[:, 6:8, :],
                        scalar1=2.0, scalar2=-1.0, op0=_ALU.mult,
                        op1=_ALU.add)
                    if tt2f_ins is not None:
                        add_dep_helper(bi_ts.ins, tt2f_ins, sync=False,
                                       reason="TS_b after TT2_f")
                tt = t12p[d].tile([128, 4, BPC], dt.bfloat16, tag=f"t12{d}")
                bi_tt1 = nc.vector.tensor_tensor(out=tt[:], in0=sg[:, 0:4, :],
                                                 in1=cg_st[d][:],
                                                 op=_ALU.mult)
                cgn = cgp[d].tile([128, 4, BPC], dt.bfloat16, tag=f"cg{d}")
                bi_tt2 = nc.vector.tensor_tensor(out=cgn[:, 0:2, :],
                                                 in0=tt[:, 0:2, :],
                                                 in1=tt[:, 2:4, :],
                                                 op=_ALU.add)
                if d == "f":
                    tt2f_ins = bi_tt2.ins
                if d == "b" and hf_ins is not None:
                    # force the DVE static order [.., TS_b, h_f, TT1_b, ..]:
                    # h_f is on the f-direction critical cycle and must not
                    # queue behind b's TT ops.
                    add_dep_helper(bi_tt1.ins, hf_ins, sync=False,
                                   reason="TT1_b after h_f")
                tcn = tcp[d].tile([128, HK, BPC], dt.bfloat16, tag=f"tc{d}")
                bi_tc = nc.scalar.activation(tcn[:], cgn[:, 0:2, :], _FT.Tanh)
                if d == "f":
                    tcf_ins = bi_tc.ins
                hn = hstp[d].tile([128, HK, BPC], dt.bfloat16, tag=f"h{d}")
                bi_h = nc.vector.tensor_tensor(out=hn[:], in0=sg[:, 4:6, :],
                                               in1=tcn[:], op=_ALU.mult)
                if d == "f":
                    hf_ins = bi_h.ins
                h_st[d] = hn
                cg_st[d] = cgn
                if t % 4 == 3:
                    del banks[d][t // 4]
            # emb tanh deferred one step: queues behind the next step's
            # sigmoids on ACT, so it can never head-block them
            while len(pend_tanh) > 1:
                ptp, pg, pk, pgi = pend_tanh.pop(0)
                gather_tanh(ptp, pg, pk)
                if pk == 2:
                    del gbuf[pgi]

        # ================= decoder =================
        # initial decoder hidden = [h_f | h_b] chunks; copy out of the
        # recurrence pools before closing them.
        for ptp, pg, pk, pgi in pend_tanh:
            gather_tanh(ptp, pg, pk)
        pend_tanh = []
        hall = const.tile([128, DK, C + 1, BPC], dt.bfloat16, tag="hall")
        nc.vector.tensor_copy(hall[:, 0:HK, 0, :], h_st["f"][:])
        nc.vector.tensor_copy(hall[:, HK:DK, 0, :], h_st["b"][:])
        rec_ctx.close()

        ceT = dec.tile([128, EK, C], dt.bfloat16, tag="ceT")
        nc.vector.memset(ceT[:, EK - 1, :], 0.0)
        nc.vector.memset(ceT[BIAS_ROW:BIAS_ROW + 1, EK - 1, :], 1.0)

        dps = ctx.enter_context(tc.tile_pool(name="dps", bufs=1, space="PSUM"))
        dps2 = ctx.enter_context(tc.tile_pool(name="dps2", bufs=2, space="PSUM"))
        dsb = ctx.enter_context(tc.tile_pool(name="dsb", bufs=2))

        for k in range(EK):
            lo = k * 128
            w = min(E, lo + 128) - lo
            tp = dps2.tile([128, C], dt.bfloat16, space="PSUM", tag="ctp")
            nc.tensor.transpose(out=tp[0:w, :], in_=ce[:, lo:lo + w],
                                identity=ident[0:C, 0:C])
            nc.scalar.activation(ceT[0:w, k, :], tp[0:w, :], _FT.Tanh)

        gxd_ps = dps.tile([128, DG, C], dt.float32, space="PSUM", tag="gxd")
        first = None
        for m in range(DG):
            for k in range(EK):
                last = (m == DG - 1 and k == EK - 1)
                bi = nc.tensor.matmul(out=gxd_ps[:, m, :],
                                      lhsT=dwih_sb[:, k, m, :],
                                      rhs=ceT[:, k, :], start=(first is None),
                                      stop=last)
                if first is None:
                    first = bi.ins
                else:
                    add_dep_helper(bi.ins, first, sync=False,
                                   reason="psum bank single-start order")
        gxd = dec.tile([128, DG, C], dt.float32, tag="gxds")
        nc.vector.tensor_copy(gxd[:], gxd_ps[:])

        for t in range(C):
            gh = dps2.tile([128, DG, BPC], dt.float32, space="PSUM", tag="gh")
            first = None
            for m in range(DG):
                for k in range(DK):
                    last = (m == DG - 1 and k == DK - 1)
                    bi = nc.tensor.matmul(out=gh[:, m, :],
                                          lhsT=dwhh_sb[:, k, m, :],
                                          rhs=hall[:, k, t, :],
                                          start=(first is None), stop=last)
                    if first is None:
                        first = bi.ins
                    else:
                        add_dep_helper(bi.ins, first, sync=False,
                                       reason="psum bank single-start order")
            pre_rz = dsb.tile([128, 8, BPC], dt.float32, tag="prerz")
            nc.vector.tensor_tensor(
                out=pre_rz[:], in0=gh[:, 0:8, :],
                in1=gxd[:, 0:8, t:t + 1].to_broadcast([128, 8, BPC]),
                op=_ALU.add)
            sig_rz = dsb.tile([128, 8, BPC], dt.float32, tag="sigrz")
            nc.scalar.activation(sig_rz[:], pre_rz[:], _FT.Sigmoid)
            hn2 = dsb.tile([128, DK, BPC], dt.float32, tag="hn2")
            nc.vector.tensor_tensor(
                out=hn2[:], in0=gh[:, 8:12, :],
                in1=bhhn_sb[:].to_broadcast([128, DK, BPC]),
                op=_ALU.add)
            tn = dsb.tile([128, DK, BPC], dt.float32, tag="tn")
            nc.vector.tensor_tensor(out=tn[:], in0=sig_rz[:, 0:4, :],
                                    in1=hn2[:], op=_ALU.mult)
            npre = dsb.tile([128, DK, BPC], dt.float32, tag="npre")
            nc.vector.tensor_tensor(
                out=npre[:], in0=tn[:],
                in1=gxd[:, 8:12, t:t + 1].to_broadcast([128, DK, BPC]),
                op=_ALU.add)
            nt_ = dsb.tile([128, DK, BPC], dt.float32, tag="nt")
            nc.scalar.activation(nt_[:], npre[:], _FT.Tanh)
            u = dsb.tile([128, DK, BPC], dt.float32, tag="u")
            nc.vector.tensor_tensor(out=u[:], in0=hall[:, :, t, :], in1=nt_[:],
                                    op=_ALU.subtract)
            v = dsb.tile([128, DK, BPC], dt.float32, tag="v")
            nc.vector.tensor_tensor(out=v[:], in0=sig_rz[:, 4:8, :], in1=u[:],
                                    op=_ALU.mult)
            w2 = dsb.tile([128, DK, BPC], dt.float32, tag="w2")
            nc.vector.tensor_tensor(out=w2[:], in0=nt_[:], in1=v[:],
                                    op=_ALU.add)
            nc.scalar.activation(hall[:, :, t + 1, :], w2[:], _FT.Tanh)

        # projection: pp[m] = sum_k pw[k,m].T @ hall[:,k,1:,:]
        pp = dps.tile([128, PK, C * BPC], dt.float32, space="PSUM", tag="pp")
        first = None
        for m in range(PK):
            for k in range(DK):
                last = (m == PK - 1 and k == DK - 1)
                bi = nc.tensor.matmul(
                    out=pp[:, m, :], lhsT=pw_sb[:, k, m, :],
                    rhs=hall[:, k, 1:C + 1, :], start=(first is None),
                    stop=last)
                if first is None:
                    first = bi.ins
                else:
                    add_dep_helper(bi.ins, first, sync=False,
                                   reason="psum bank single-start order")
        pbt = dec.tile([128, PK, C * BPC], dt.bfloat16, tag="pbt")
        for m in range(PK):
            nc.scalar.activation(pbt[:, m, :], pp[:, m, :], _FT.Identity,
                                 bias=pb_sb[:, m:m + 1])
        lg_ps = dps.tile([128, 2], dt.float32, space="PSUM", tag="lg")
        NPB = C * BPC
        for k in range(PK):
            nc.tensor.matmul(out=lg_ps[0:NPB, :], lhsT=pbt[:, k, :],
                             rhs=cw_sb[:, k, :], start=(k == 0),
                             stop=(k == PK - 1))
        lgs = dsb.tile([128, 2], dt.float32, tag="lgs")
        nc.vector.tensor_tensor(out=lgs[0:NPB, :], in0=lg_ps[0:NPB, :],
                                in1=cb_sb[0:NPB, :], op=_ALU.add)
        mx = dsb.tile([128, 1], dt.float32, tag="mx")
        nc.vector.tensor_reduce(out=mx[0:NPB, :], in_=lgs[0:NPB, :],
                                axis=mybir.AxisListType.X,
                                op=_ALU.max)
        nmx = dsb.tile([128, 1], dt.float32, tag="nmx")
        nc.vector.tensor_scalar_mul(nmx[0:NPB, :], mx[0:NPB, :], -1.0)
        ex = dsb.tile([128, 2], dt.float32, tag="ex")
        nc.scalar.activation(ex[0:NPB, :], lgs[0:NPB, :], _FT.Exp,
                             bias=nmx[0:NPB, :1])
        sm = dsb.tile([128, 1], dt.float32, tag="sm")
        nc.vector.tensor_reduce(out=sm[0:NPB, :], in_=ex[0:NPB, :],
                                axis=mybir.AxisListType.X,
                                op=_ALU.add)
        ls = dsb.tile([128, 1], dt.float32, tag="ls")
        nc.scalar.activation(ls[0:NPB, :], sm[0:NPB, :], _FT.Ln)
        ntot = dsb.tile([128, 1], dt.float32, tag="ntot")
        nc.vector.tensor_tensor(out=ntot[0:NPB, :], in0=nmx[0:NPB, :],
                                in1=ls[0:NPB, :], op=_ALU.subtract)
        out_sb = dsb.tile([128, 2], dt.float32, tag="out")
        nc.scalar.activation(out_sb[0:NPB, :], lgs[0:NPB, :], _FT.Identity,
                             bias=ntot[0:NPB, :1])
        nc.sync.dma_start(out=y[:], in_=out_sb[0:NPB, :])

    nc.compile()
    return nc


def _prep_host(inputs, s_steps):
    """Host-side packing of weights/indices into the kernel's tile layouts."""
    SS = s_steps
    # gate chunk order (f, i, o, g); g rows scaled by 2 for the
    # tanh g = 2*sigmoid(2g) - 1 trick.
    perm = np.r_[H:2 * H, 0:H, 3 * H:4 * H, 2 * H:3 * H]
    gscale = np.ones((4 * H, 1), dtype=F32)
    gscale[3 * H:] = 2.0

    def lstm_pack(pre, scale_g):
        sc = gscale if scale_g else 1.0
        Wih = np.asarray(inputs[f"{pre}_Wih"], F32)[perm] * sc
        Whh = np.asarray(inputs[f"{pre}_Whh"], F32)[perm] * sc
        bias = ((np.asarray(inputs[f"{pre}_bih"], F32) +
                 np.asarray(inputs[f"{pre}_bhh"], F32))[perm]
                * (sc[:, 0] if scale_g else 1.0))
        wihT = _aug_wihT(Wih, bias, G4)
        whhT = _pack_kxm(Whh.T.astype(F32), HK, G4)
        return wihT, whhT

    # forward uses a real tanh for the g gate (no 2x prescale); backward
    # uses the 2*sigmoid(2g)-1 trick.
    wih_f, whh_f = lstm_pack("f", False)
    wih_b, whh_b = lstm_pack("b", True)

    d_Wih = np.asarray(inputs["d_Wih"], F32)
    d_Whh = np.asarray(inputs["d_Whh"], F32)
    d_bih = np.asarray(inputs["d_bih"], F32)
    d_bhh = np.asarray(inputs["d_bhh"], F32)
    dbias = d_bih.copy()
    dbias[:4 * H] += d_bhh[:4 * H]  # r,z gate biases fold; n keeps only bih
    dwih = _aug_wihT(d_Wih, dbias, DG)
    dwhh = _pack_kxm(d_Whh.T.astype(F32), DK, DG)
    bhhn = np.ascontiguousarray(
        d_bhh[4 * H:].reshape(DK, 128).T.reshape(128, DK, 1).astype(F32))

    proj_W = np.asarray(inputs["proj_W"], F32)
    proj_b = np.asarray(inputs["proj_b"], F32)
    cls_W = np.asarray(inputs["cls_W"], F32)
    cls_b = np.asarray(inputs["cls_b"], F32)
    pw = _pack_kxm(proj_W.T, DK, PK)
    pbt = np.ascontiguousarray(proj_b.reshape(PK, 128).T.astype(F32))
    cwt = np.ascontiguousarray(
        cls_W.T.reshape(PK, 128, 2).transpose(1, 0, 2).astype(BF16))
    cbt = np.ascontiguousarray(np.broadcast_to(cls_b, (128, 2)).astype(F32))

    emb = np.asarray(inputs["embed_W"], F32).astype(BF16)
    ecw = np.asarray(inputs["embed_class_W"], F32).astype(BF16)
    clsi = np.asarray(inputs["classes"]).astype(np.int32).reshape(C, 1)

    seq = np.asarray(inputs["seq"]).astype(np.int32)
    shared = dict(emb=emb, wih_f=wih_f, whh_f=whh_f, wih_b=wih_b, whh_b=whh_b,
                  dwih=dwih, dwhh=dwhh, bhhn=bhhn, pw=pw, pb=pbt, cw=cwt,
                  cb=cbt, ecw=ecw, clsi=clsi)
    in_maps = []
    NT = SS * BPC // 128
    for cix in range(NCORES):
        sl = seq[cix * BPC:(cix + 1) * BPC, :SS]       # [16, SS]
        seqi = np.ascontiguousarray(sl.T.reshape(NT, 128).T.astype(np.int32))
        m = dict(shared)
        m["seqi"] = seqi
        in_maps.append(m)
    return in_maps


LAST_EXEC_NS = None
LAST_TRACE = None


def kernel(**inputs) -> np.ndarray:
    global LAST_EXEC_NS, LAST_TRACE
    s_steps = int(os.environ.get("KERNEL_S_STEPS", S))
    if s_steps not in _BUILD_CACHE:
        _BUILD_CACHE[s_steps] = _build_program(s_steps)
    nc = _BUILD_CACHE[s_steps]
    in_maps = _prep_host(inputs, s_steps)
    trace = os.environ.get("KERNEL_PROFILE") == "1"
    kw = {}
    if trace:
        from concourse import bass_utils as _bu
        _bu.upload_artifacts = lambda tmpdir: tmpdir  # zero-egress container
        kw = dict(trace=True, tmpdir=os.environ.get("KERNEL_TRACE_DIR"))
    res = run_bass_kernel_spmd(nc, in_maps, list(range(NCORES)), **kw)
    if res.exec_time_ns is not None:
        LAST_EXEC_NS = res.exec_time_ns
        LAST_TRACE = (res.instructions_and_trace[1]
                      if res.instructions_and_trace else None)
    out = np.empty((C, B, 2), dtype=F32)
    for cix in range(NCORES):
        out[:, cix * BPC:(cix + 1) * BPC, :] = \
            res.results[cix]["y"].reshape(C, BPC, 2)
    return out
